# revision 6
# baseline (speedup 1.0000x reference)
"""CLIP (InfoNCE) loss kernel v3 for Trainium2, 8 NeuronCores.

loss = 0.5*(ce_m + ce_s), ce_m = mean_i(LSE_j l[i,:] - l[i,i]), l = scale*(m@s.T)

Structure (per core, rows r = core*2048 + [0,2048)):
  - main matmul: either bf16 (4 MMs of [K128,N512] per [128,1024] tile,
    k-chunk accumulated) or fp8e4m3 DoubleRow (2 MMs, K=256 in one shot).
    Measured on HW both stream ~1 element/cycle; bf16 has lower per-MM
    overhead, fp8 halves DMA + SBUF.
  - drain of each [128,1024] PSUM logits tile, split ACT/DVE by tile:
      ACT: e_bf16 = exp(scale*l - 87.9898)                   (~1061 ns)
      DVE: bits_i16 = round(max(l*128*log2e*scale, 0)) whose bf16
           reinterpretation is 2^(scale*l*log2e - 127) up to the
           Schraudolph mantissa ripple (mean 1.0406835, folded into the
           ACT bias / host log constant)                     (~1180 ns)
  - per-tile row partial sums via DVE reduce_sum (fast-mode, ~219 ns);
    NO accum_out anywhere (accum_out doubles the ACT op cost on HW).
  - column sums via ones-matmuls into 4 concurrent 32-col PE strips,
    two column-groups per PSUM bank (free halves), one [97,512] copy per
    pair, strided DMA out at the end.
  - diag, final row reduction and the LSE/loss merge happen on host (f64).
"""

import math
from contextlib import ExitStack

import numpy as np
import ml_dtypes

import concourse.bacc as bacc
import concourse.bass as bass
import concourse.tile as tile
from concourse import mybir
from concourse.bass_utils import run_bass_kernel_spmd

FP8 = ml_dtypes.float8_e4m3
BF16 = ml_dtypes.bfloat16

B = 16384
D = 256
NCORES = 8
ROWS = B // NCORES          # 2048 rows per core
P = 128
MT = ROWS // P              # 16 m-tiles
PN = 512                    # psum bank width (f32)
GW = 2                      # panels per group -> [128, 1024] drains
GN = B // (PN * GW)         # 16 column-groups
NQ = 8                      # s DMA chunks (early-start pipelining)
QW = B // NQ
CS = 2                      # column-sum split per panel (4 PE strips)
NSTRIP = GW * CS
W = PN // CS                # 256 columns per strip

LOG2E = 1.4426950408889634
# mean of (1+f)*2^-f over uniform f: the Schraudolph ripple of the DVE path
RIPPLE = 1.0406835250031703
LN_RIPPLE = math.log(RIPPLE)
LN2_127 = 127.0 * math.log(2.0)          # 88.02969193111305

f32 = mybir.dt.float32
bf16 = mybir.dt.bfloat16
i16 = mybir.dt.int16
fp8 = mybir.dt.float8e4

_nc_cache: dict = {}


def _build(scale: float, kd: int = 0, mmdt: str = "fp8",
           ebufs: int = 10, mainbufs: int = 3, colbufs: int = 2,
           loop_reps: int = 1, loads_in_loop: int = 1, nocol: int = 0,
           colmode: str = "4way", redw: int = 0, drainmode: str = "full",
           copyeng: str = "dve") -> "bass.Bass":
    nc = bacc.Bacc(trn_type="TRN2")

    mdt, MDT = (fp8, FP8) if mmdt == "fp8" else (bf16, BF16)
    m_d = nc.dram_tensor("m_in", [P, 2, ROWS], mdt, kind="ExternalInput")
    s_d = nc.dram_tensor("s_in", [P, 2, B], mdt, kind="ExternalInput")

    rowsums_d = nc.dram_tensor("rowsums", [P, MT * GN], f32, kind="ExternalOutput")
    colsum_d = nc.dram_tensor("colsum", [NSTRIP, GN, PN], f32, kind="ExternalOutput")

    act_bias = -(LN2_127 - LN_RIPPLE)            # exp(scale*l + act_bias)
    dve_c1 = 128.0 * LOG2E * scale               # bits = round(max(l*c1, 0))

    _dve_sets = [
        {(i * 16 // kd + g) % 16 for i in range(kd)} if kd else set()
        for g in range(GN)
    ]

    def is_dve(g, mt):
        return mt in _dve_sets[g]

    with ExitStack() as ctx:
        tc = ctx.enter_context(tile.TileContext(nc))
        singles = ctx.enter_context(tc.tile_pool(name="singles", bufs=1))
        epool = ctx.enter_context(tc.tile_pool(name="epool", bufs=ebufs))
        mainps = ctx.enter_context(tc.tile_pool(name="mainps", bufs=mainbufs, space="PSUM"))
        colps = ctx.enter_context(tc.tile_pool(name="colps", bufs=colbufs, space="PSUM"))

        def emit_loads():
            m_sb = singles.tile([P, 2, ROWS], mdt, tag="m_sb")
            nc.sync.dma_start(out=m_sb[:, :, :], in_=m_d[:, :, :])
            s_sb = singles.tile([P, 2, B], mdt, tag="s_sb")
            for q in range(NQ):
                nc.sync.dma_start(
                    out=s_sb[:, :, q * QW:(q + 1) * QW],
                    in_=s_d[:, :, q * QW:(q + 1) * QW],
                )
            return m_sb, s_sb

        if not loads_in_loop:
            m_sb, s_sb = emit_loads()
        if loop_reps > 1:
            ctx.enter_context(tc.For_i(0, loop_reps, 1, staggered_reset=True))
        if loads_in_loop:
            m_sb, s_sb = emit_loads()

        ones = singles.tile([P, 1], bf16, tag="ones")
        nc.vector.memset(ones, 1.0)
        negb = singles.tile([P, 1], f32, tag="negb")
        nc.vector.memset(negb, act_bias)

        rowsums_sb = singles.tile([P, MT * GN], f32, tag="rowsums")
        if not nocol:
            colsb = singles.tile([32 * (NSTRIP - 1) + 1, GN, PN], f32, tag="colsb")

        for g in range(GN):
            colpsum = None if nocol else colps.tile([32 * (NSTRIP - 1) + 1, PN], f32)
            if True:
                for mt in range(MT):
                    ps = mainps.tile([P, GW * PN], f32)  # 2 banks
                    if mmdt == "fp8":
                        for sub in range(GW):
                            n0 = g * GW * PN + sub * PN
                            nc.tensor.matmul(
                                ps[:, sub * PN:(sub + 1) * PN],
                                lhsT=m_sb[:, :, mt * P:(mt + 1) * P],
                                rhs=s_sb[:, :, n0:n0 + PN],
                                start=True, stop=True,
                                perf_mode=mybir.MatmulPerfMode.DoubleRow,
                            )
                    else:
                        # k outer so each weight tile loads once per (g,mt)
                        for k in range(2):
                            for sub in range(GW):
                                n0 = g * GW * PN + sub * PN
                                nc.tensor.matmul(
                                    ps[:, sub * PN:(sub + 1) * PN],
                                    lhsT=m_sb[:, k, mt * P:(mt + 1) * P],
                                    rhs=s_sb[:, k, n0:n0 + PN],
                                    start=(k == 0), stop=(k == 1),
                                )
                    slot = mt * GN + g
                    if drainmode == "none":
                        # PE-only probe: tiny PSUM read keeps MMs alive
                        nc.vector.reduce_sum(
                            rowsums_sb[:, slot:slot + 1], ps[:, 0:64],
                            axis=mybir.AxisListType.X,
                        )
                        continue
                    if is_dve(g, mt):
                        e_i = epool.tile([P, GW * PN], i16, tag="e")
                        nc.vector.tensor_scalar(
                            out=e_i, in0=ps, scalar1=dve_c1, scalar2=0.0,
                            op0=mybir.AluOpType.mult, op1=mybir.AluOpType.max,
                        )
                        e = e_i.bitcast(bf16)
                    else:
                        e_b = epool.tile([P, GW * PN], bf16, tag="e")
                        nc.scalar.activation(
                            e_b, ps, mybir.ActivationFunctionType.Exp,
                            bias=negb[:, 0:1], scale=scale,
                        )
                        e = e_b
                    if nocol:
                        pass
                    elif colmode == "8way":
                        # 2 row-groups (K=64) x 4 col strips; K=64 partials
                        # in different free halves, summed on host.
                        for r2 in range(2):
                            for sub in range(GW):
                                for ci in range(CS):
                                    strip = sub * CS + ci
                                    nc.tensor.matmul(
                                        colpsum[32 * strip:32 * strip + 1,
                                                r2 * W:(r2 + 1) * W],
                                        lhsT=ones[64 * r2:64 * (r2 + 1), 0:1],
                                        rhs=e[64 * r2:64 * (r2 + 1),
                                              sub * PN + ci * W: sub * PN + (ci + 1) * W],
                                        start=(mt == 0),
                                        stop=(mt == MT - 1),
                                        tile_position=(64 * r2, 32 * strip),
                                    )
                    else:
                        # 4 col strips, K=128; host reads only r2=0 half
                        for sub in range(GW):
                            for ci in range(CS):
                                strip = sub * CS + ci
                                nc.tensor.matmul(
                                    colpsum[32 * strip:32 * strip + 1, 0:W],
                                    lhsT=ones,
                                    rhs=e[:, sub * PN + ci * W: sub * PN + (ci + 1) * W],
                                    start=(mt == 0),
                                    stop=(mt == MT - 1),
                                    tile_position=(0, 32 * strip),
                                )
                    # reduce emitted AFTER the strips: decouples the DVE read
                    # of e from ACT's write of it (-19% measured)
                    nc.vector.reduce_sum(
                        rowsums_sb[:, slot:slot + 1],
                        e[:, 0:redw] if redw else e,
                        axis=mybir.AxisListType.X,
                    )
            if not nocol:
                if copyeng == "act":
                    nc.scalar.copy(out=colsb[:, g, :], in_=colpsum)
                else:
                    nc.vector.tensor_copy(out=colsb[:, g, :], in_=colpsum)

        if not nocol:
            for strip in range(NSTRIP):
                nc.sync.dma_start(
                    out=colsum_d[strip, :, :],
                    in_=colsb[32 * strip:32 * strip + 1, :, :],
                )
        nc.sync.dma_start(out=rowsums_d[:, :], in_=rowsums_sb)

    nc.compile()
    return nc


def _get_nc(scale: float, **kw):
    key = (scale, tuple(sorted(kw.items())))
    if key not in _nc_cache:
        _nc_cache[key] = _build(scale, **kw)
    return _nc_cache[key]


def make_in_maps(inputs: dict, mmdt: str = "fp8") -> list[dict]:
    m = np.asarray(inputs["modality_features"], dtype=np.float32)
    s = np.asarray(inputs["sequence_features"], dtype=np.float32)
    assert m.shape == (B, D) and s.shape == (B, D)
    MDT = FP8 if mmdt == "fp8" else BF16

    # [P, 2, B]: partition p, k-chunk o -> feature dim k = o*128 + p
    sT = np.ascontiguousarray(s.T.reshape(2, P, B).transpose(1, 0, 2)).astype(MDT)
    in_maps = []
    for c in range(NCORES):
        r = slice(c * ROWS, (c + 1) * ROWS)
        mT = np.ascontiguousarray(
            m[r].T.reshape(2, P, ROWS).transpose(1, 0, 2)
        ).astype(MDT)
        in_maps.append({"m_in": mT, "s_in": sT})
    return in_maps


def build_for_bench(inputs: dict, loop_reps: int = 1, **kw):
    scale = float(np.asarray(inputs["logit_scale"], dtype=np.float32))
    return _build(scale, loop_reps=loop_reps, **kw)


def _merge(results, inputs: dict, kd: int, colmode: str = "4way"):
    m = np.asarray(inputs["modality_features"], dtype=np.float64)
    s = np.asarray(inputs["sequence_features"], dtype=np.float64)
    scale = float(np.asarray(inputs["logit_scale"], dtype=np.float32))

    # ACT-tile sums are exactly e^{scale*l}*2^-127*RIPPLE; DVE-tile sums the
    # same on average. One shared log-constant undoes it.
    log_unit = LN2_127 - LN_RIPPLE

    # rowsums_sb[p, mt*GN+g]: row index within core = mt*128+p
    rowsum = np.concatenate([
        r["rowsums"].astype(np.float64).reshape(P, MT, GN).sum(axis=2)
        .T.reshape(-1)
        for r in results
    ])

    colsum = np.zeros(B, dtype=np.float64)
    for r in results:
        cs = r["colsum"].astype(np.float64)  # [NSTRIP, GN, 2*W]
        csr = cs.reshape(GW, CS, GN, 2, W)
        # [s, g, r2*W+j]: 8way: K=64 partials summed over r2; 4way: r2=0 only
        arr = (csr.sum(axis=3) if colmode == "8way" else csr[:, :, :, 0, :])
        colsum += arr.transpose(2, 0, 1, 3).reshape(B)

    rowlse = np.log(rowsum) + log_unit
    collse = np.log(colsum) + log_unit
    diag = scale * np.einsum("ij,ij->i", m, s)
    loss = np.mean(0.5 * (rowlse + collse) - diag)
    return np.asarray(loss, dtype=np.float32)


def run(inputs: dict, trace: bool = False, **kw):
    scale = float(np.asarray(inputs["logit_scale"], dtype=np.float32))
    nc = _get_nc(scale, **kw)
    in_maps = make_in_maps(inputs, mmdt=kw.get("mmdt", "fp8"))
    res = run_bass_kernel_spmd(nc, in_maps, list(range(NCORES)), trace=trace)
    return _merge(res.results, inputs, kw.get("kd", 0),
                  colmode=kw.get("colmode", "4way")), res


def kernel(**inputs) -> np.ndarray:
    out, _ = run(inputs, trace=False)
    return out
